# revision 2
# baseline (speedup 1.0000x reference)
"""Trainium2 Bass kernel for nn_CrossAttention (B=4, NQ=NK=1024, D=1024, H=16).

Sharding: 8 cores = 4 batches x 2 head-groups (8 heads each). Per core:
  - inputs arrive pre-transposed/sliced on host (free): xqT/xkT/xvT [D, T] fp16,
    Wq/Wk/Wv column slices [D, 512] fp16, Wo row slice [512, D] fp16.
  - projections produce Q^T/K^T per head-pair [128, T] (lhsT = W slice, rhs = xT)
  - scores computed transposed (scoresT [Tk, Tq]) as 4-way tile_position-packed
    quads (2 heads x 2 M-halves, K=64 each) into a single 2-bank PSUM tile;
    ONE fused exp activation per quad (free size 1024) halves ScalarE overhead.
  - exp stream starts during the projections: quads for pairs 0-2 (query chunk 0)
    are interleaved into the projection chains so ScalarE ramps at ~15us.
  - denominators via an augmented ones-column in V (row 64 of the PV output),
    reciprocal via 2-op approx, gpsimd partition_broadcast, DVE mul
  - per query-chunk (512 queries) pipeline: after all 4 pairs' PV+norm for a
    chunk, the out-projection + fp16 output DMA for that chunk overlap the next
    chunk's scores/PV, shrinking the serial tail.
  - host sums the two head-group partials per batch and adds the bias.
All matmuls fp16 (1 cycle/row on PE), accumulation fp32 in PSUM.
"""
import sys

sys.path.insert(0, "/opt/trn_rl_repo")

from contextlib import ExitStack

import numpy as np

import concourse.bass as bass
import concourse.tile as tile
from concourse import bacc, mybir
from concourse.bass_utils import run_bass_kernel_spmd

F32 = mybir.dt.float32
F16 = mybir.dt.float16

B, NQ, NK, D, H, HD = 4, 1024, 1024, 1024, 16, 64
NCORES = 8
HPC = 8          # heads per core
F = HPC * HD     # 512: per-core projection width
KT = D // 128    # 8 k-tiles over D
PAIRS = HPC // 2  # 4 head pairs
TKT = NK // 128  # 8 tiles over key tokens
NCH = NQ // 512  # 2 query chunks


def _emit(tc):
    nc = tc.nc
    ctx = ExitStack()

    xqT = nc.dram_tensor("xqT", [D, NQ], F16, kind="ExternalInput").ap()
    xkT = nc.dram_tensor("xkT", [D, NK], F16, kind="ExternalInput").ap()
    xvT = nc.dram_tensor("xvT", [D, NK], F16, kind="ExternalInput").ap()
    wq = nc.dram_tensor("wq", [D, F], F16, kind="ExternalInput").ap()
    wk = nc.dram_tensor("wk", [D, F], F16, kind="ExternalInput").ap()
    wv = nc.dram_tensor("wv", [D, F], F16, kind="ExternalInput").ap()
    wo = nc.dram_tensor("wo", [F, D], F16, kind="ExternalInput").ap()
    out = nc.dram_tensor("out", [NQ, D], F16, kind="ExternalOutput").ap()

    wpool = ctx.enter_context(tc.tile_pool(name="wpool", bufs=1))
    qkv = ctx.enter_context(tc.tile_pool(name="qkv", bufs=1))
    xpool = ctx.enter_context(tc.tile_pool(name="xpool", bufs=24))
    expool = ctx.enter_context(tc.tile_pool(name="expool", bufs=26))
    psum = ctx.enter_context(tc.tile_pool(name="psum", bufs=1, space="PSUM"))
    nrm = ctx.enter_context(tc.tile_pool(name="nrm", bufs=2))
    ost = ctx.enter_context(tc.tile_pool(name="ost", bufs=3))

    # zero-bias AP for activations (avoids the const-page TENSOR_LOAD)
    zb = nrm.tile([128, 1], F32, tag="zb", bufs=1)
    nc.vector.memset(zb[:], 0.0)

    # ---- phase-1 input DMAs: only what QK projections need ----
    wq_sb = wpool.tile([128, KT, F], F16, tag="wq")
    wk_sb = wpool.tile([128, KT, F], F16, tag="wk")
    wv_sb = wpool.tile([128, KT, F], F16, tag="wv")
    wo_sb = wpool.tile([128, PAIRS, D], F16, tag="wo")
    for k in range(KT):
        nc.gpsimd.dma_start(out=wq_sb[:, k, :], in_=wq[k * 128:(k + 1) * 128, :])
        nc.scalar.dma_start(out=wk_sb[:, k, :], in_=wk[k * 128:(k + 1) * 128, :])
    xq_t, xk_t, xv_t = [], [], []
    for k in range(KT):
        t = xpool.tile([128, NQ], F16, tag="x", name=f"xq{k}")
        nc.sync.dma_start(out=t[:], in_=xqT[k * 128:(k + 1) * 128, :])
        xq_t.append(t)
        t = xpool.tile([128, NK], F16, tag="x", name=f"xk{k}")
        nc.sync.dma_start(out=t[:], in_=xkT[k * 128:(k + 1) * 128, :])
        xk_t.append(t)

    # ---- persistent intermediates ----
    qt = [qkv.tile([128, NQ], F16, tag=f"qt{p}", name=f"qt{p}") for p in range(PAIRS)]
    kt = [qkv.tile([128, NK], F16, tag=f"kt{p}", name=f"kt{p}") for p in range(PAIRS)]
    vp_sb = qkv.tile([128, TKT, HPC, HD + 1], F16, tag="vp")  # V + ones col
    att = [qkv.tile([128, NQ], F16, tag=f"att{p}", name=f"att{p}") for p in range(PAIRS)]
    nc.vector.memset(vp_sb[:, :, :, HD:HD + 1], 1.0)

    scale = 1.0 / float(np.sqrt(HD))
    ex = {}

    def emit_quad(p, c, tkm):
        """One scoresT quad (2 heads x 128 keys x 512 queries) + fused exp."""
        ps = psum.tile([128, 2, 512], F32, tag="quad", bufs=1,
                       name=f"qps_{p}_{c}_{tkm}")
        et = expool.tile([128, 2, 512], F16, tag="ex", name=f"ex_{p}_{c}_{tkm}")
        for hh in range(2):
            r0 = hh * 64
            for mh in range(2):
                c0 = mh * 64
                nc.tensor.matmul(
                    out=ps[c0:c0 + 64, hh, :],
                    lhsT=kt[p][r0:r0 + 64, tkm * 128 + c0:tkm * 128 + c0 + 64],
                    rhs=qt[p][r0:r0 + 64, c * 512:(c + 1) * 512],
                    start=True, stop=True,
                    tile_position=(r0, c0))
        nc.scalar.activation(out=et[:], in_=ps[:],
                             func=mybir.ActivationFunctionType.Exp,
                             scale=scale, bias=zb[:])
        ex[(p, c, tkm)] = et

    # early quads: pairs 0-2, chunk 0, interleaved into projection chains below
    early = [(p, 0, k) for p in range(3) for k in range(TKT)]
    ei = 0

    def pop_quads(n, ready_pairs):
        nonlocal ei
        while n > 0 and ei < len(early):
            p, c, k = early[ei]
            if p >= ready_pairs:
                return
            emit_quad(p, c, k)
            ei += 1
            n -= 1

    # ---- QK projections, pair-major; early quads between chains ----
    chain_idx = 0
    for m in range(PAIRS):
        for x_t, w_sb, dst in ((xq_t, wq_sb, qt[m]), (xk_t, wk_sb, kt[m])):
            for n in range(NCH):
                pp = psum.tile([128, 512], F32, tag="mm", bufs=2,
                               name=f"ps_p{m}_{n}")
                for k in range(KT):
                    nc.tensor.matmul(out=pp[:],
                                     lhsT=w_sb[:, k, m * 128:(m + 1) * 128],
                                     rhs=x_t[k][:, n * 512:(n + 1) * 512],
                                     start=(k == 0), stop=(k == KT - 1))
                nc.vector.tensor_copy(out=dst[:, n * 512:(n + 1) * 512], in_=pp[:])
                chain_idx += 1
                if chain_idx > 4:
                    pop_quads(1, ready_pairs=chain_idx // 4)

    # ---- phase-2 DMAs ----
    for k in range(KT):
        nc.gpsimd.dma_start(out=wv_sb[:, k, :], in_=wv[k * 128:(k + 1) * 128, :])
        t = xpool.tile([128, NK], F16, tag="x", name=f"xv{k}")
        nc.sync.dma_start(out=t[:], in_=xvT[k * 128:(k + 1) * 128, :])
        xv_t.append(t)
    for p in range(PAIRS):
        nc.scalar.dma_start(out=wo_sb[:, p, :], in_=wo[p * 128:(p + 1) * 128, :])

    # ---- V projection; remaining early quads between chains ----
    for t in range(TKT):
        psv = psum.tile([128, 512], F32, tag="mm", bufs=2, name=f"psv_{t}")
        for k in range(KT):
            nc.tensor.matmul(out=psv[:],
                             lhsT=xv_t[k][:, t * 128:(t + 1) * 128],
                             rhs=wv_sb[:, k, :], start=(k == 0),
                             stop=(k == KT - 1))
        nc.vector.tensor_copy(
            out=vp_sb[:, t, :, 0:HD],
            in_=psv[:].rearrange("p (h d) -> p h d", h=HPC))
        pop_quads(2, ready_pairs=3)

    def emit_norm(p, c, pv0, pv1):
        """Per-(pair, chunk) softmax normalization from the PV ones-row."""
        cs = slice(c * 512, (c + 1) * 512)
        den = nrm.tile([33, 512], F32, tag="den", name=f"den_{p}_{c}")
        nc.vector.memset(den[:], 1.0)
        nc.vector.tensor_copy(out=den[0:1, :], in_=pv0[64:65, :])
        nc.vector.tensor_copy(out=den[32:33, :], in_=pv1[64:65, :])
        rscr = nrm.tile([33, 512], F32, tag="rscr", name=f"rscr_{p}_{c}", bufs=1)
        rec = nrm.tile([33, 512], F32, tag="rec", name=f"rec_{p}_{c}")
        nc.vector.reciprocal_approx_accurate(out=rec[:], in_=den[:], scratch=rscr[:])
        # HW partition_broadcast reads physical partition 0, so move head 1's
        # reciprocal row down first
        rec1 = nrm.tile([1, 512], F32, tag="rec1", name=f"rec1_{p}_{c}")
        nc.sync.dma_start(out=rec1[:], in_=rec[32:33, :])
        rb0 = nrm.tile([64, 512], F32, tag="rb", name=f"rb0_{p}_{c}")
        nc.gpsimd.partition_broadcast(out_ap=rb0[:], in_ap=rec[0:1, :], channels=64)
        nc.vector.tensor_mul(out=att[p][0:64, cs], in0=pv0[0:64, :], in1=rb0[:])
        rb1 = nrm.tile([64, 512], F32, tag="rb", name=f"rb1_{p}_{c}")
        nc.gpsimd.partition_broadcast(out_ap=rb1[:], in_ap=rec1[:], channels=64)
        tmp = nrm.tile([64, 512], F16, tag="tmp", name=f"tmp_{p}_{c}")
        nc.vector.tensor_mul(out=tmp[:], in0=pv1[0:64, :], in1=rb1[:])
        nc.sync.dma_start(out=att[p][64:128, cs], in_=tmp[:])

    def emit_out(c):
        """Out-projection + fp16 DMA for one query chunk (overlaps next chunk)."""
        for qi in range(4):
            q = c * 4 + qi
            for half in range(2):
                pso = psum.tile([128, 512], F32, tag="mm", bufs=2,
                                name=f"pso_{q}_{half}")
                for p4 in range(PAIRS):
                    nc.tensor.matmul(out=pso[:],
                                     lhsT=att[p4][:, q * 128:(q + 1) * 128],
                                     rhs=wo_sb[:, p4, half * 512:(half + 1) * 512],
                                     start=(p4 == 0), stop=(p4 == PAIRS - 1))
                ot = ost.tile([128, 512], F16, tag="ot", name=f"ot_{q}_{half}")
                nc.vector.tensor_copy(out=ot[:], in_=pso[:])
                eng = nc.sync if (qi + half) % 2 == 0 else nc.gpsimd
                eng.dma_start(out=out[q * 128:(q + 1) * 128,
                                      half * 512:(half + 1) * 512], in_=ot[:])

    # ---- pipelined (pair, chunk) iterations ----
    seq = [(p, c) for c in range(NCH) for p in range(PAIRS)]
    # quads for seq[i] were emitted 3 iterations earlier; emit seq[i+3] here
    for i, (p, c) in enumerate(seq):
        nxt = seq[i + 3] if i + 3 < len(seq) else None
        pv0 = psum.tile([65, 512], F32, tag="pv", bufs=4, name=f"pv0_{p}_{c}")
        pv1 = psum.tile([65, 512], F32, tag="pv", bufs=4, name=f"pv1_{p}_{c}")
        for k in range(TKT):
            if nxt is not None:
                emit_quad(nxt[0], nxt[1], k)
            et = ex[(p, c, k)]
            for hh, pv in ((0, pv0), (1, pv1)):
                h = p * 2 + hh
                nc.tensor.matmul(out=pv[:],
                                 lhsT=vp_sb[:, k, h, :],
                                 rhs=et[:, hh, :],
                                 start=(k == 0), stop=(k == TKT - 1))
            del ex[(p, c, k)]
        emit_norm(p, c, pv0, pv1)
        # out-projection for chunk 0 one iteration late (lets norm(3,0) drain)
        if (p, c) == (0, 1):
            emit_out(0)
    emit_out(1)
    ctx.close()


_NC_CACHE = None


def build():
    global _NC_CACHE
    if _NC_CACHE is None:
        nc = bacc.Bacc("TRN2", target_bir_lowering=False, debug=False,
                       num_devices=NCORES)
        with tile.TileContext(nc) as tc:
            _emit(tc)
        nc.compile()
        _NC_CACHE = nc
    return _NC_CACHE


def make_in_maps(inputs):
    q = np.asarray(inputs["query_tokens"], dtype=np.float32)
    kk = np.asarray(inputs["key_tokens"], dtype=np.float32)
    v = np.asarray(inputs["value_tokens"], dtype=np.float32)
    Wq = np.asarray(inputs["Wq"], dtype=np.float32)
    Wk = np.asarray(inputs["Wk"], dtype=np.float32)
    Wv = np.asarray(inputs["Wv"], dtype=np.float32)
    Wo = np.asarray(inputs["Wo"], dtype=np.float32)

    qT = [np.ascontiguousarray(q[b].T).astype(np.float16) for b in range(B)]
    kT = [np.ascontiguousarray(kk[b].T).astype(np.float16) for b in range(B)]
    vT = [np.ascontiguousarray(v[b].T).astype(np.float16) for b in range(B)]
    wq_g = [np.ascontiguousarray(Wq[:, g * F:(g + 1) * F]).astype(np.float16)
            for g in range(2)]
    wk_g = [np.ascontiguousarray(Wk[:, g * F:(g + 1) * F]).astype(np.float16)
            for g in range(2)]
    wv_g = [np.ascontiguousarray(Wv[:, g * F:(g + 1) * F]).astype(np.float16)
            for g in range(2)]
    wo_g = [np.ascontiguousarray(Wo[g * F:(g + 1) * F, :]).astype(np.float16)
            for g in range(2)]

    in_maps = []
    for c in range(NCORES):
        b, g = c // 2, c % 2
        in_maps.append({
            "xqT": qT[b], "xkT": kT[b], "xvT": vT[b],
            "wq": wq_g[g], "wk": wk_g[g], "wv": wv_g[g], "wo": wo_g[g],
        })
    return in_maps


def combine(results, bo):
    out = np.zeros((B, NQ, D), dtype=np.float32)
    for c in range(NCORES):
        out[c // 2] += results[c]["out"].astype(np.float32)
    out += np.asarray(bo, dtype=np.float32)[None, None, :]
    return out


def kernel(**inputs):
    nc = build()
    in_maps = make_in_maps(inputs)
    res = run_bass_kernel_spmd(nc, in_maps, list(range(NCORES)))
    return combine(res.results, inputs["bo"])


# revision 8
# speedup vs baseline: 1.3130x; 1.3130x over previous
"""Trainium2 Bass kernel for nn_CrossAttention (B=4, NQ=NK=1024, D=1024, H=16).

Sharding: 8 cores = 4 batches x 2 head-groups (8 heads each). Per core:
  - inputs arrive pre-transposed/sliced on host (free): xqT/xkT/xvT [D, T] fp16,
    Wq/Wk/Wv column slices [D, 512] fp16, Wo row slice [512, D] fp16.
  - projections produce Q^T/K^T per head-pair [128, T] (lhsT = W slice, rhs = xT)
  - scores computed transposed (scoresT [Tk, Tq]) as 4-way tile_position-packed
    quads (2 heads x 2 M-halves, K=64 each) into a single 2-bank PSUM tile;
    ONE fused exp activation per quad (free size 1024) halves ScalarE overhead.
  - exp stream starts during the projections: quads for pairs 0-2 (query chunk 0)
    are interleaved into the projection chains so ScalarE ramps at ~15us.
  - denominators via an augmented ones-column in V (row 64 of the PV output),
    reciprocal via 2-op approx, gpsimd partition_broadcast, DVE mul
  - per query-chunk (512 queries) pipeline: after all 4 pairs' PV+norm for a
    chunk, the out-projection + fp16 output DMA for that chunk overlap the next
    chunk's scores/PV, shrinking the serial tail.
  - host sums the two head-group partials per batch and adds the bias.
All matmuls fp16 (1 cycle/row on PE), accumulation fp32 in PSUM.
"""
import sys

sys.path.insert(0, "/opt/trn_rl_repo")

from contextlib import ExitStack

import numpy as np

import concourse.bass as bass
import concourse.tile as tile
from concourse import bacc, mybir
from concourse.bass_utils import run_bass_kernel_spmd

F32 = mybir.dt.float32
F16 = mybir.dt.float16

B, NQ, NK, D, H, HD = 4, 1024, 1024, 1024, 16, 64
NCORES = 8
HPC = 8          # heads per core
F = HPC * HD     # 512: per-core projection width
KT = D // 128    # 8 k-tiles over D
PAIRS = HPC // 2  # 4 head pairs
TKT = NK // 128  # 8 tiles over key tokens
NCH = NQ // 512  # 2 query chunks


def _emit(tc):
    nc = tc.nc
    ctx = ExitStack()

    xqT = nc.dram_tensor("xqT", [D, NQ], F16, kind="ExternalInput").ap()
    xkT = nc.dram_tensor("xkT", [D, NK], F16, kind="ExternalInput").ap()
    xvT = nc.dram_tensor("xvT", [D, NK], F16, kind="ExternalInput").ap()
    wq = nc.dram_tensor("wq", [D, F], F16, kind="ExternalInput").ap()
    wk = nc.dram_tensor("wk", [D, F], F16, kind="ExternalInput").ap()
    wv = nc.dram_tensor("wv", [D, F], F16, kind="ExternalInput").ap()
    wo = nc.dram_tensor("wo", [F, D], F16, kind="ExternalInput").ap()
    out = nc.dram_tensor("out", [NQ, D], F16, kind="ExternalOutput").ap()

    wpool = ctx.enter_context(tc.tile_pool(name="wpool", bufs=1))
    qkv = ctx.enter_context(tc.tile_pool(name="qkv", bufs=1))
    xpool = ctx.enter_context(tc.tile_pool(name="xpool", bufs=24))
    expool = ctx.enter_context(tc.tile_pool(name="expool", bufs=26))
    psum = ctx.enter_context(tc.tile_pool(name="psum", bufs=1, space="PSUM"))
    nrm = ctx.enter_context(tc.tile_pool(name="nrm", bufs=2))
    ost = ctx.enter_context(tc.tile_pool(name="ost", bufs=3))

    # zero-bias AP for activations (avoids the const-page TENSOR_LOAD)
    zb = nrm.tile([128, 1], F32, tag="zb", bufs=1)
    nc.vector.memset(zb[:], 0.0)

    # ---- phase-1 input DMAs (QK projection inputs), x split over 2 queues ----
    wq_sb = wpool.tile([128, KT, F], F16, tag="wq")
    wk_sb = wpool.tile([128, KT, F], F16, tag="wk")
    wv_sb = wpool.tile([128, KT, F], F16, tag="wv")
    wo_sb = wpool.tile([128, PAIRS, D], F16, tag="wo")
    for k in range(KT):
        nc.gpsimd.dma_start(out=wq_sb[:, k, :], in_=wq[k * 128:(k + 1) * 128, :])
        nc.gpsimd.dma_start(out=wk_sb[:, k, :], in_=wk[k * 128:(k + 1) * 128, :])
    xq_t, xk_t, xv_t = [], [], []
    for k in range(KT):
        t = xpool.tile([128, NQ], F16, tag="x", name=f"xq{k}")
        nc.sync.dma_start(out=t[:], in_=xqT[k * 128:(k + 1) * 128, :])
        xq_t.append(t)
        t = xpool.tile([128, NK], F16, tag="x", name=f"xk{k}")
        nc.scalar.dma_start(out=t[:], in_=xkT[k * 128:(k + 1) * 128, :])
        xk_t.append(t)

    # ---- persistent intermediates ----
    qt = [qkv.tile([128, NQ], F16, tag=f"qt{p}", name=f"qt{p}") for p in range(PAIRS)]
    kt = [qkv.tile([128, NK], F16, tag=f"kt{p}", name=f"kt{p}") for p in range(PAIRS)]
    vp_sb = qkv.tile([128, TKT, HPC, HD + 1], F16, tag="vp")  # V + ones col
    att = [qkv.tile([128, NQ], F16, tag=f"att{p}", name=f"att{p}") for p in range(PAIRS)]
    nc.vector.memset(vp_sb[:, :, :, HD:HD + 1], 1.0)

    scale = 1.0 / float(np.sqrt(HD))
    ex = {}

    def emit_quad(p, c, tkm):
        """One scoresT quad (2 heads x 128 keys x 512 queries) + fused exp."""
        ps = psum.tile([128, 2, 512], F32, tag="quad", bufs=2,
                       name=f"qps_{p}_{c}_{tkm}")
        et = expool.tile([128, 2, 512], F16, tag="ex", name=f"ex_{p}_{c}_{tkm}")
        for hh in range(2):
            r0 = hh * 64
            for mh in range(2):
                c0 = mh * 64
                nc.tensor.matmul(
                    out=ps[c0:c0 + 64, hh, :],
                    lhsT=kt[p][r0:r0 + 64, tkm * 128 + c0:tkm * 128 + c0 + 64],
                    rhs=qt[p][r0:r0 + 64, c * 512:(c + 1) * 512],
                    start=True, stop=True,
                    tile_position=(r0, c0))
        nc.scalar.activation(out=et[:], in_=ps[:],
                             func=mybir.ActivationFunctionType.Exp,
                             scale=scale, bias=zb[:])
        ex[(p, c, tkm)] = et

    # early quads: pairs 0-2, chunk 0, interleaved into projection chains below
    early = [(p, 0, k) for p in range(3) for k in range(TKT)]
    ei = 0

    def pop_quads(n, ready_pairs):
        nonlocal ei
        while n > 0 and ei < len(early):
            p, c, k = early[ei]
            if p >= ready_pairs:
                return
            emit_quad(p, c, k)
            ei += 1
            n -= 1

    # ---- QK projections: k-outer within each pair's 4 chains, so the PE
    # tracks the DMA frontier instead of serializing behind a stalled chain
    for m in range(PAIRS):
        chains = []  # (psum, w_sb, x_t, n)
        for x_t, w_sb in ((xq_t, wq_sb), (xk_t, wk_sb)):
            for n in range(NCH):
                pp = psum.tile([128, 512], F32, tag="ps", bufs=4,
                               name=f"ps_p{m}_{len(chains)}")
                chains.append((pp, w_sb, x_t, n))
        for k in range(KT):
            for pp, w_sb, x_t, n in chains:
                nc.tensor.matmul(out=pp[:],
                                 lhsT=w_sb[:, k, m * 128:(m + 1) * 128],
                                 rhs=x_t[k][:, n * 512:(n + 1) * 512],
                                 start=(k == 0), stop=(k == KT - 1))
        for ci, (pp, w_sb, x_t, n) in enumerate(chains):
            dst = qt[m] if ci < 2 else kt[m]
            nc.vector.tensor_copy(out=dst[:, n * 512:(n + 1) * 512], in_=pp[:])
            if m >= 1:
                pop_quads(1, ready_pairs=m)

    # ---- phase-2 DMAs ----
    for k in range(KT):
        nc.gpsimd.dma_start(out=wv_sb[:, k, :], in_=wv[k * 128:(k + 1) * 128, :])
        t = xpool.tile([128, NK], F16, tag="x", name=f"xv{k}")
        nc.sync.dma_start(out=t[:], in_=xvT[k * 128:(k + 1) * 128, :])
        xv_t.append(t)
    for p in range(PAIRS):
        nc.scalar.dma_start(out=wo_sb[:, p, :], in_=wo[p * 128:(p + 1) * 128, :])

    # ---- V projection, k-outer in two groups of 4 ----
    for g in range(2):
        chains = []
        for t in range(g * 4, g * 4 + 4):
            psv = psum.tile([128, 512], F32, tag="ps", bufs=4, name=f"psv_{t}")
            chains.append((psv, t))
        for k in range(KT):
            for psv, t in chains:
                nc.tensor.matmul(out=psv[:],
                                 lhsT=xv_t[k][:, t * 128:(t + 1) * 128],
                                 rhs=wv_sb[:, k, :], start=(k == 0),
                                 stop=(k == KT - 1))
        for psv, t in chains:
            nc.vector.tensor_copy(
                out=vp_sb[:, t, :, 0:HD],
                in_=psv[:].rearrange("p (h d) -> p h d", h=HPC))
            pop_quads(3, ready_pairs=3)

    def emit_norm(p, c, pv0, pv1):
        """Per-(pair, chunk) softmax normalization from the PV ones-row."""
        cs = slice(c * 512, (c + 1) * 512)
        den = nrm.tile([33, 512], F32, tag="den", name=f"den_{p}_{c}")
        nc.vector.memset(den[:], 1.0)
        nc.vector.tensor_copy(out=den[0:1, :], in_=pv0[64:65, :])
        nc.vector.tensor_copy(out=den[32:33, :], in_=pv1[64:65, :])
        rscr = nrm.tile([33, 512], F32, tag="rscr", name=f"rscr_{p}_{c}", bufs=1)
        rec = nrm.tile([33, 512], F32, tag="rec", name=f"rec_{p}_{c}")
        nc.vector.reciprocal_approx_accurate(out=rec[:], in_=den[:], scratch=rscr[:])
        # HW partition_broadcast reads physical partition 0, so move head 1's
        # reciprocal row down first
        rec1 = nrm.tile([1, 512], F32, tag="rec1", name=f"rec1_{p}_{c}")
        nc.sync.dma_start(out=rec1[:], in_=rec[32:33, :])
        rb0 = nrm.tile([64, 512], F32, tag="rb", name=f"rb0_{p}_{c}")
        nc.gpsimd.partition_broadcast(out_ap=rb0[:], in_ap=rec[0:1, :], channels=64)
        nc.vector.tensor_mul(out=att[p][0:64, cs], in0=pv0[0:64, :], in1=rb0[:])
        rb1 = nrm.tile([64, 512], F32, tag="rb", name=f"rb1_{p}_{c}")
        nc.gpsimd.partition_broadcast(out_ap=rb1[:], in_ap=rec1[:], channels=64)
        tmp = nrm.tile([64, 512], F16, tag="tmp", name=f"tmp_{p}_{c}")
        nc.vector.tensor_mul(out=tmp[:], in0=pv1[0:64, :], in1=rb1[:])
        nc.sync.dma_start(out=att[p][64:128, cs], in_=tmp[:])

    def emit_out(c):
        """Out-projection + fp16 DMA for one query chunk (overlaps next chunk)."""
        for qi in range(4):
            q = c * 4 + qi
            for half in range(2):
                pso = psum.tile([128, 512], F32, tag="ps", bufs=4,
                                name=f"pso_{q}_{half}")
                for p4 in range(PAIRS):
                    nc.tensor.matmul(out=pso[:],
                                     lhsT=att[p4][:, q * 128:(q + 1) * 128],
                                     rhs=wo_sb[:, p4, half * 512:(half + 1) * 512],
                                     start=(p4 == 0), stop=(p4 == PAIRS - 1))
                ot = ost.tile([128, 512], F16, tag="ot", name=f"ot_{q}_{half}")
                nc.vector.tensor_copy(out=ot[:], in_=pso[:])
                eng = nc.sync if (qi + half) % 2 == 0 else nc.gpsimd
                eng.dma_start(out=out[q * 128:(q + 1) * 128,
                                      half * 512:(half + 1) * 512], in_=ot[:])

    # ---- pipelined (pair, chunk) iterations ----
    seq = [(p, c) for c in range(NCH) for p in range(PAIRS)]
    # quads for seq[i] were emitted 3 iterations earlier; emit seq[i+3] here
    for i, (p, c) in enumerate(seq):
        nxt = seq[i + 3] if i + 3 < len(seq) else None
        pv0 = psum.tile([65, 512], F32, tag="ps", bufs=4, name=f"pv0_{p}_{c}",
                        padded_shape=[128, 512])
        pv1 = psum.tile([65, 512], F32, tag="ps", bufs=4, name=f"pv1_{p}_{c}",
                        padded_shape=[128, 512])
        for k in range(TKT):
            if nxt is not None:
                emit_quad(nxt[0], nxt[1], k)
            et = ex[(p, c, k)]
            for hh, pv in ((0, pv0), (1, pv1)):
                h = p * 2 + hh
                nc.tensor.matmul(out=pv[:],
                                 lhsT=vp_sb[:, k, h, :],
                                 rhs=et[:, hh, :],
                                 start=(k == 0), stop=(k == TKT - 1))
            del ex[(p, c, k)]
        emit_norm(p, c, pv0, pv1)
        # out-projection for chunk 0 one iteration late (lets norm(3,0) drain)
        if (p, c) == (0, 1):
            emit_out(0)
    emit_out(1)
    ctx.close()


_NC_CACHE = None


def build():
    global _NC_CACHE
    if _NC_CACHE is None:
        nc = bacc.Bacc("TRN2", target_bir_lowering=False, debug=False,
                       num_devices=NCORES)
        with tile.TileContext(nc) as tc:
            _emit(tc)
        nc.compile()
        _NC_CACHE = nc
    return _NC_CACHE


def make_in_maps(inputs):
    q = np.asarray(inputs["query_tokens"], dtype=np.float32)
    kk = np.asarray(inputs["key_tokens"], dtype=np.float32)
    v = np.asarray(inputs["value_tokens"], dtype=np.float32)
    Wq = np.asarray(inputs["Wq"], dtype=np.float32)
    Wk = np.asarray(inputs["Wk"], dtype=np.float32)
    Wv = np.asarray(inputs["Wv"], dtype=np.float32)
    Wo = np.asarray(inputs["Wo"], dtype=np.float32)

    qT = [np.ascontiguousarray(q[b].T).astype(np.float16) for b in range(B)]
    kT = [np.ascontiguousarray(kk[b].T).astype(np.float16) for b in range(B)]
    vT = [np.ascontiguousarray(v[b].T).astype(np.float16) for b in range(B)]
    wq_g = [np.ascontiguousarray(Wq[:, g * F:(g + 1) * F]).astype(np.float16)
            for g in range(2)]
    wk_g = [np.ascontiguousarray(Wk[:, g * F:(g + 1) * F]).astype(np.float16)
            for g in range(2)]
    wv_g = [np.ascontiguousarray(Wv[:, g * F:(g + 1) * F]).astype(np.float16)
            for g in range(2)]
    wo_g = [np.ascontiguousarray(Wo[g * F:(g + 1) * F, :]).astype(np.float16)
            for g in range(2)]

    in_maps = []
    for c in range(NCORES):
        b, g = c // 2, c % 2
        in_maps.append({
            "xqT": qT[b], "xkT": kT[b], "xvT": vT[b],
            "wq": wq_g[g], "wk": wk_g[g], "wv": wv_g[g], "wo": wo_g[g],
        })
    return in_maps


def combine(results, bo):
    out = np.zeros((B, NQ, D), dtype=np.float32)
    for c in range(NCORES):
        out[c // 2] += results[c]["out"].astype(np.float32)
    out += np.asarray(bo, dtype=np.float32)[None, None, :]
    return out


def kernel(**inputs):
    nc = build()
    in_maps = make_in_maps(inputs)
    res = run_bass_kernel_spmd(nc, in_maps, list(range(NCORES)))
    return combine(res.results, inputs["bo"])


# revision 18
# speedup vs baseline: 1.3223x; 1.0071x over previous
"""Trainium2 Bass kernel for nn_CrossAttention (B=4, NQ=NK=1024, D=1024, H=16).

Sharding: 8 cores = 4 batches x 2 head-groups (8 heads each). Per core:
  - inputs arrive pre-transposed/sliced on host (free): xqT/xkT/xvT [D, T] fp16,
    Wq/Wk/Wv column slices [D, 512] fp16, Wo row slice [512, D] fp16.
  - projections produce Q^T/K^T per head-pair [128, T] (lhsT = W slice, rhs = xT)
  - scores computed transposed (scoresT [Tk, Tq]) as 4-way tile_position-packed
    quads (2 heads x 2 M-halves, K=64 each) into a single 2-bank PSUM tile;
    ONE fused exp activation per quad (free size 1024) halves ScalarE overhead.
  - exp stream starts during the projections: quads for pairs 0-2 (query chunk 0)
    are interleaved into the projection chains so ScalarE ramps at ~15us.
  - denominators via an augmented ones-column in V (row 64 of the PV output),
    reciprocal via 2-op approx, gpsimd partition_broadcast, DVE mul
  - per query-chunk (512 queries) pipeline: after all 4 pairs' PV+norm for a
    chunk, the out-projection + fp16 output DMA for that chunk overlap the next
    chunk's scores/PV, shrinking the serial tail.
  - host sums the two head-group partials per batch and adds the bias.
All matmuls fp16 (1 cycle/row on PE), accumulation fp32 in PSUM.
"""
import sys

sys.path.insert(0, "/opt/trn_rl_repo")

from contextlib import ExitStack

import numpy as np

import concourse.bass as bass
import concourse.tile as tile
from concourse import bacc, mybir
from concourse.bass_utils import run_bass_kernel_spmd

F32 = mybir.dt.float32
F16 = mybir.dt.float16

B, NQ, NK, D, H, HD = 4, 1024, 1024, 1024, 16, 64
NCORES = 8
HPC = 8          # heads per core
F = HPC * HD     # 512: per-core projection width
KT = D // 128    # 8 k-tiles over D
PAIRS = HPC // 2  # 4 head pairs
TKT = NK // 128  # 8 tiles over key tokens
NCH = NQ // 512  # 2 query chunks


def _emit(tc):
    nc = tc.nc
    ctx = ExitStack()

    xqT = nc.dram_tensor("xqT", [D, NQ], F16, kind="ExternalInput").ap()
    xkT = nc.dram_tensor("xkT", [D, NK], F16, kind="ExternalInput").ap()
    xvT = nc.dram_tensor("xvT", [D, NK], F16, kind="ExternalInput").ap()
    # wq/wk host-swizzled to [m-pair][sbuf-partition][k-tile*128]: one
    # contiguous 256KB DMA per pair so pair-0 weights land first
    wq = nc.dram_tensor("wq", [PAIRS * 128, D], F16, kind="ExternalInput").ap()
    wk = nc.dram_tensor("wk", [PAIRS * 128, D], F16, kind="ExternalInput").ap()
    wv = nc.dram_tensor("wv", [D, F], F16, kind="ExternalInput").ap()
    wo = nc.dram_tensor("wo", [F, D], F16, kind="ExternalInput").ap()
    out = nc.dram_tensor("out", [NQ, D], F16, kind="ExternalOutput").ap()

    wpool = ctx.enter_context(tc.tile_pool(name="wpool", bufs=1))
    qkv = ctx.enter_context(tc.tile_pool(name="qkv", bufs=1))
    xpool = ctx.enter_context(tc.tile_pool(name="xpool", bufs=24))
    expool = ctx.enter_context(tc.tile_pool(name="expool", bufs=26))
    psum = ctx.enter_context(tc.tile_pool(name="psum", bufs=1, space="PSUM"))
    nrm = ctx.enter_context(tc.tile_pool(name="nrm", bufs=2))
    ost = ctx.enter_context(tc.tile_pool(name="ost", bufs=3))

    # zero-bias AP for activations (avoids the const-page TENSOR_LOAD)
    zb = nrm.tile([128, 1], F32, tag="zb", bufs=1)
    nc.vector.memset(zb[:], 0.0)

    # ---- phase-1 input DMAs: one queue each for xq / xk / weights, with
    # pair-0's weight blocks first so pair-0 projections finish earliest
    wq_sb = wpool.tile([128, PAIRS, KT, 128], F16, tag="wq")
    wk_sb = wpool.tile([128, PAIRS, KT, 128], F16, tag="wk")
    wv_sb = wpool.tile([128, KT, F], F16, tag="wv")
    wo_sb = wpool.tile([128, PAIRS, D], F16, tag="wo")
    for m in range(PAIRS):
        nc.scalar.dma_start(out=wq_sb[:, m, :, :], in_=wq[m * 128:(m + 1) * 128, :])
        nc.scalar.dma_start(out=wk_sb[:, m, :, :], in_=wk[m * 128:(m + 1) * 128, :])
    xq_t, xk_t, xv_t = [], [], []
    for k in range(KT):
        t = xpool.tile([128, NQ], F16, tag="x", name=f"xq{k}")
        nc.sync.dma_start(out=t[:], in_=xqT[k * 128:(k + 1) * 128, :])
        xq_t.append(t)
        t = xpool.tile([128, NK], F16, tag="x", name=f"xk{k}")
        nc.gpsimd.dma_start(out=t[:], in_=xkT[k * 128:(k + 1) * 128, :])
        xk_t.append(t)

    # ---- persistent intermediates ----
    qt = [qkv.tile([128, NQ], F16, tag=f"qt{p}", name=f"qt{p}") for p in range(PAIRS)]
    kt = [qkv.tile([128, NK], F16, tag=f"kt{p}", name=f"kt{p}") for p in range(PAIRS)]
    vp_sb = qkv.tile([128, TKT, HPC, HD + 1], F16, tag="vp")  # V + ones col
    att = [qkv.tile([128, NQ], F16, tag=f"att{p}", name=f"att{p}") for p in range(PAIRS)]
    nc.vector.memset(vp_sb[:, :, :, HD:HD + 1], 1.0)

    scale = 1.0 / float(np.sqrt(HD))
    ex = {}

    def emit_quad(p, c, tkm):
        """One scoresT quad (2 heads x 128 keys x 512 queries) + fused exp."""
        ps = psum.tile([128, 2, 512], F32, tag="quad", bufs=2,
                       name=f"qps_{p}_{c}_{tkm}")
        et = expool.tile([128, 2, 512], F16, tag="ex", name=f"ex_{p}_{c}_{tkm}")
        for hh in range(2):
            r0 = hh * 64
            for mh in range(2):
                c0 = mh * 64
                nc.tensor.matmul(
                    out=ps[c0:c0 + 64, hh, :],
                    lhsT=kt[p][r0:r0 + 64, tkm * 128 + c0:tkm * 128 + c0 + 64],
                    rhs=qt[p][r0:r0 + 64, c * 512:(c + 1) * 512],
                    start=True, stop=True,
                    tile_position=(r0, c0))
        nc.scalar.activation(out=et[:], in_=ps[:],
                             func=mybir.ActivationFunctionType.Exp,
                             scale=scale, bias=zb[:])
        ex[(p, c, tkm)] = et

    # early quads: pairs 0-2, chunk 0, interleaved into projection chains below
    early = [(p, 0, k) for p in range(3) for k in range(TKT)]
    ei = 0

    def pop_quads(n, ready_pairs):
        nonlocal ei
        while n > 0 and ei < len(early):
            p, c, k = early[ei]
            if p >= ready_pairs:
                return
            emit_quad(p, c, k)
            ei += 1
            n -= 1

    # ---- QK projections: k-outer within each pair's 4 chains, so the PE
    # tracks the DMA frontier instead of serializing behind a stalled chain
    for m in range(PAIRS):
        chains = []  # (psum, w_sb, x_t, n)
        for x_t, w_sb in ((xq_t, wq_sb), (xk_t, wk_sb)):
            for n in range(NCH):
                pp = psum.tile([128, 512], F32, tag="ps", bufs=4,
                               name=f"ps_p{m}_{len(chains)}")
                chains.append((pp, w_sb, x_t, n))
        for k in range(KT):
            for pp, w_sb, x_t, n in chains:
                nc.tensor.matmul(out=pp[:],
                                 lhsT=w_sb[:, m, k, :],
                                 rhs=x_t[k][:, n * 512:(n + 1) * 512],
                                 start=(k == 0), stop=(k == KT - 1))
        for ci, (pp, w_sb, x_t, n) in enumerate(chains):
            dst = qt[m] if ci < 2 else kt[m]
            nc.vector.tensor_copy(out=dst[:, n * 512:(n + 1) * 512], in_=pp[:])
            if m >= 1:
                pop_quads(1, ready_pairs=m)

    # ---- phase-2 DMAs (wo on gpsimd so it doesn't bubble the exp stream) ----
    for k in range(KT):
        nc.gpsimd.dma_start(out=wv_sb[:, k, :], in_=wv[k * 128:(k + 1) * 128, :])
        t = xpool.tile([128, NK], F16, tag="x", name=f"xv{k}")
        nc.sync.dma_start(out=t[:], in_=xvT[k * 128:(k + 1) * 128, :])
        xv_t.append(t)
    for p in range(PAIRS):
        nc.gpsimd.dma_start(out=wo_sb[:, p, :], in_=wo[p * 128:(p + 1) * 128, :])

    # ---- V projection, k-outer in two groups of 4 ----
    for g in range(2):
        chains = []
        for t in range(g * 4, g * 4 + 4):
            psv = psum.tile([128, 512], F32, tag="ps", bufs=4, name=f"psv_{t}")
            chains.append((psv, t))
        for k in range(KT):
            for psv, t in chains:
                nc.tensor.matmul(out=psv[:],
                                 lhsT=xv_t[k][:, t * 128:(t + 1) * 128],
                                 rhs=wv_sb[:, k, :], start=(k == 0),
                                 stop=(k == KT - 1))
        for psv, t in chains:
            nc.vector.tensor_copy(
                out=vp_sb[:, t, :, 0:HD],
                in_=psv[:].rearrange("p (h d) -> p h d", h=HPC))
            pop_quads(3, ready_pairs=3)

    def emit_norm(p, c, pv0, pv1):
        """Per-(pair, chunk) softmax normalization from the PV ones-row.

        DVE copies the PV PSUM tiles to SBUF fp16 immediately, releasing
        the shared PSUM ring slots ~4us earlier than the end of the
        reciprocal/broadcast chain (which otherwise stalls later PV and
        out-projection matmuls behind the ring)."""
        cs = slice(c * 512, (c + 1) * 512)
        pvs0 = nrm.tile([65, 512], F16, tag="pvs0", name=f"pvs0_{p}_{c}")
        pvs1 = nrm.tile([65, 512], F16, tag="pvs1", name=f"pvs1_{p}_{c}")
        nc.vector.tensor_copy(out=pvs0[:], in_=pv0[:])
        nc.vector.tensor_copy(out=pvs1[:], in_=pv1[:])
        den = nrm.tile([33, 512], F32, tag="den", name=f"den_{p}_{c}")
        nc.vector.memset(den[:], 1.0)
        nc.vector.tensor_copy(out=den[0:1, :], in_=pvs0[64:65, :])
        nc.vector.tensor_copy(out=den[32:33, :], in_=pvs1[64:65, :])
        rscr = nrm.tile([33, 512], F32, tag="rscr", name=f"rscr_{p}_{c}", bufs=1)
        rec = nrm.tile([33, 512], F32, tag="rec", name=f"rec_{p}_{c}")
        nc.vector.reciprocal_approx_accurate(out=rec[:], in_=den[:], scratch=rscr[:])
        # HW partition_broadcast reads physical partition 0, so move head 1's
        # reciprocal row down first
        rec1 = nrm.tile([1, 512], F32, tag="rec1", name=f"rec1_{p}_{c}")
        nc.sync.dma_start(out=rec1[:], in_=rec[32:33, :])
        rb0 = nrm.tile([64, 512], F32, tag="rb", name=f"rb0_{p}_{c}")
        nc.gpsimd.partition_broadcast(out_ap=rb0[:], in_ap=rec[0:1, :], channels=64)
        nc.vector.tensor_mul(out=att[p][0:64, cs], in0=pvs0[0:64, :], in1=rb0[:])
        rb1 = nrm.tile([64, 512], F32, tag="rb", name=f"rb1_{p}_{c}")
        nc.gpsimd.partition_broadcast(out_ap=rb1[:], in_ap=rec1[:], channels=64)
        tmp = nrm.tile([64, 512], F16, tag="tmp", name=f"tmp_{p}_{c}")
        nc.vector.tensor_mul(out=tmp[:], in0=pvs1[0:64, :], in1=rb1[:])
        nc.sync.dma_start(out=att[p][64:128, cs], in_=tmp[:])

    def emit_out(c, wide=False):
        """Out-projection + fp16 DMA for one query chunk.

        wide=True (final chunk): all 8 output tiles accumulate p4-major across
        8 PSUM banks (4 ring slots + 2 idle quad slots), so the p4=0..2 ranks
        run while the last pair's normalization is still draining."""
        tiles = [(c * 4 + qi, half) for qi in range(4) for half in range(2)]
        if wide:
            psos = []
            for j in range(2):
                qa = psum.tile([128, 2, 512], F32, tag="quad", bufs=2,
                               name=f"oq_{c}_{j}")
                psos += [qa[:, 0, :], qa[:, 1, :]]
            for j in range(4):
                psos.append(psum.tile([128, 512], F32, tag="ps", bufs=4,
                                      name=f"os_{c}_{j}"))
            for p4 in range(PAIRS):
                for (q, half), pso in zip(tiles, psos):
                    nc.tensor.matmul(out=pso,
                                     lhsT=att[p4][:, q * 128:(q + 1) * 128],
                                     rhs=wo_sb[:, p4, half * 512:(half + 1) * 512],
                                     start=(p4 == 0), stop=(p4 == PAIRS - 1))
            for i, ((q, half), pso) in enumerate(zip(tiles, psos)):
                ot = ost.tile([128, 512], F16, tag="ot", name=f"ot_{q}_{half}")
                nc.vector.tensor_copy(out=ot[:], in_=pso)
                eng = nc.sync if i % 2 == 0 else nc.gpsimd
                eng.dma_start(out=out[q * 128:(q + 1) * 128,
                                      half * 512:(half + 1) * 512], in_=ot[:])
            return
        for i, (q, half) in enumerate(tiles):
            pso = psum.tile([128, 512], F32, tag="ps", bufs=4,
                            name=f"pso_{q}_{half}")
            for p4 in range(PAIRS):
                nc.tensor.matmul(out=pso[:],
                                 lhsT=att[p4][:, q * 128:(q + 1) * 128],
                                 rhs=wo_sb[:, p4, half * 512:(half + 1) * 512],
                                 start=(p4 == 0), stop=(p4 == PAIRS - 1))
            ot = ost.tile([128, 512], F16, tag="ot", name=f"ot_{q}_{half}")
            nc.vector.tensor_copy(out=ot[:], in_=pso[:])
            eng = nc.sync if i % 2 == 0 else nc.gpsimd
            eng.dma_start(out=out[q * 128:(q + 1) * 128,
                                  half * 512:(half + 1) * 512], in_=ot[:])

    # ---- pipelined (pair, chunk) iterations ----
    seq = [(p, c) for c in range(NCH) for p in range(PAIRS)]
    # quads for seq[i] were emitted 3 iterations earlier; emit seq[i+3] here
    for i, (p, c) in enumerate(seq):
        nxt = seq[i + 3] if i + 3 < len(seq) else None
        pv0 = psum.tile([65, 512], F32, tag="ps", bufs=4, name=f"pv0_{p}_{c}",
                        padded_shape=[128, 512])
        pv1 = psum.tile([65, 512], F32, tag="ps", bufs=4, name=f"pv1_{p}_{c}",
                        padded_shape=[128, 512])
        for k in range(TKT):
            if nxt is not None:
                emit_quad(nxt[0], nxt[1], k)
            et = ex[(p, c, k)]
            for hh, pv in ((0, pv0), (1, pv1)):
                h = p * 2 + hh
                nc.tensor.matmul(out=pv[:],
                                 lhsT=vp_sb[:, k, h, :],
                                 rhs=et[:, hh, :],
                                 start=(k == 0), stop=(k == TKT - 1))
            del ex[(p, c, k)]
        emit_norm(p, c, pv0, pv1)
        # out-projection for chunk 0 one iteration late (lets norm(3,0) drain)
        if (p, c) == (0, 1):
            emit_out(0)
    emit_out(1, wide=True)
    ctx.close()


_NC_CACHE = None


def build():
    global _NC_CACHE
    if _NC_CACHE is None:
        nc = bacc.Bacc("TRN2", target_bir_lowering=False, debug=False,
                       num_devices=NCORES)
        with tile.TileContext(nc) as tc:
            _emit(tc)
        nc.compile()
        _NC_CACHE = nc
    return _NC_CACHE


def make_in_maps(inputs):
    q = np.asarray(inputs["query_tokens"], dtype=np.float32)
    kk = np.asarray(inputs["key_tokens"], dtype=np.float32)
    v = np.asarray(inputs["value_tokens"], dtype=np.float32)
    Wq = np.asarray(inputs["Wq"], dtype=np.float32)
    Wk = np.asarray(inputs["Wk"], dtype=np.float32)
    Wv = np.asarray(inputs["Wv"], dtype=np.float32)
    Wo = np.asarray(inputs["Wo"], dtype=np.float32)

    def swizzle(w_cols):
        # [1024, 512] -> [pair m][sbuf partition p][k-tile k][d]: block m is a
        # contiguous [128, 1024] so one DMA per pair lands pair-major
        a = w_cols.reshape(8, 128, 4, 128).transpose(2, 1, 0, 3)
        return np.ascontiguousarray(a.reshape(512, 1024)).astype(np.float16)

    qT = [np.ascontiguousarray(q[b].T).astype(np.float16) for b in range(B)]
    kT = [np.ascontiguousarray(kk[b].T).astype(np.float16) for b in range(B)]
    vT = [np.ascontiguousarray(v[b].T).astype(np.float16) for b in range(B)]
    wq_g = [swizzle(Wq[:, g * F:(g + 1) * F]) for g in range(2)]
    wk_g = [swizzle(Wk[:, g * F:(g + 1) * F]) for g in range(2)]
    wv_g = [np.ascontiguousarray(Wv[:, g * F:(g + 1) * F]).astype(np.float16)
            for g in range(2)]
    wo_g = [np.ascontiguousarray(Wo[g * F:(g + 1) * F, :]).astype(np.float16)
            for g in range(2)]

    in_maps = []
    for c in range(NCORES):
        b, g = c // 2, c % 2
        in_maps.append({
            "xqT": qT[b], "xkT": kT[b], "xvT": vT[b],
            "wq": wq_g[g], "wk": wk_g[g], "wv": wv_g[g], "wo": wo_g[g],
        })
    return in_maps


def combine(results, bo):
    out = np.zeros((B, NQ, D), dtype=np.float32)
    for c in range(NCORES):
        out[c // 2] += results[c]["out"].astype(np.float32)
    out += np.asarray(bo, dtype=np.float32)[None, None, :]
    return out


def kernel(**inputs):
    nc = build()
    in_maps = make_in_maps(inputs)
    res = run_bass_kernel_spmd(nc, in_maps, list(range(NCORES)))
    return combine(res.results, inputs["bo"])
